# revision 10
# baseline (speedup 1.0000x reference)
"""Trainium2 Bass kernel: batched forward kinematics (nn_DiffKin).

Computes, for each batch element b and frame n:
    W[b, n] = prod_{i<=n} ( O_i @ M_i(angle_i(b)) )        (4x4 transforms)
where M_i is a revolute rotation / prismatic translation about a fixed axis.

Strategy (pure data-parallel across 8 NeuronCores, batch-major layout):
  * Host folds origins/axes/joint-types/mimic into per-frame constant
    3x4 (or 4x4) tables A' = A + C, B, C with
        L_n(b) = A'_n + u_n(b) * B_n + w_n(b) * C_n,
        u = sin(m*theta+o)  (or  m*theta+o  for prismatic),
        w = -cos(m*theta+o) = sin(m*theta+o - pi/2).
    so the only per-batch device work is two ScalarE Sin evaluations per
    frame plus elementwise tensor ops.
  * SBUF layout: partition p holds batch elements b = p*Q + q (q inner,
    contiguous) so the final DMA writes long contiguous HBM runs.
  * Per frame: GpSimd builds L (2 muls + add), VectorE runs the affine
    chain product (5 fused-broadcast tensor ops), ScalarE feeds coefficients.
  * Output staged in SBUF chunks of C frames, DMA'd out overlapped.

The program is specialized at trace time on the structural inputs (indices,
types, axes norms); batch data flows through DRAM tensors.
"""

import os
import sys

import numpy as np

for _p in ("/opt/trn_rl_repo", "/root/.axon_site/_ro/trn_rl_repo"):
    if os.path.isdir(_p) and _p not in sys.path:
        sys.path.append(_p)

import concourse.bass as bass  # noqa: E402
import concourse.tile as tile  # noqa: E402
from concourse import bacc, mybir  # noqa: E402
from concourse.bass_utils import run_bass_kernel_spmd  # noqa: E402

F32 = mybir.dt.float32
AF = mybir.ActivationFunctionType

N_CORES = 8
P = 128  # SBUF partitions
CHUNK = 8  # frames per output staging chunk

# module-level stash for test harness introspection
last_results = None
last_tables_rep = None
_program_cache = {}


# --------------------------------------------------------------------------
# Host-side specialization
# --------------------------------------------------------------------------

def _skew(a):
    x, y, z = a
    return np.array([[0.0, -z, y], [z, 0.0, -x], [-y, x, 0.0]], dtype=np.float64)


def _frame_specs(all_axes, all_origins, mimic_multipliers, mimic_offsets,
                 ctrlable_indices, mimic_dst_indices, mimic_src_indices,
                 joint_types):
    """Fold structural inputs into per-frame specs + constant tables.

    Returns (affine, frames, tables):
      affine  : True if all origin bottom rows are [0,0,0,1] (3x4 chain math)
      frames  : list of dicts per frame:
                  kind: 'rev' | 'pri' | 'const'
                  src  : source column into joint_angles (var kinds)
                  mult, off : effective angle transform (var kinds)
      tables  : np.float32 [NF, 4, 16]  (slots: A', B, C, scalars)
    """
    axes = np.asarray(all_axes, dtype=np.float64)
    origins = np.asarray(all_origins, dtype=np.float64)
    nf = origins.shape[0]
    types = np.asarray(joint_types).astype(np.int64)
    ctrl = np.asarray(ctrlable_indices).astype(np.int64)
    mdst = np.asarray(mimic_dst_indices).astype(np.int64)
    msrc = np.asarray(mimic_src_indices).astype(np.int64)
    mmul = np.asarray(mimic_multipliers, dtype=np.float64)
    moff = np.asarray(mimic_offsets, dtype=np.float64)

    bottom = origins[:, 3, :]
    affine = bool(np.all(np.abs(bottom - np.array([0.0, 0.0, 0.0, 1.0])) < 1e-6))
    ni = 3 if affine else 4

    # per-frame angle source: angle_n(b) = mult * theta[b, src] + off
    # (src=None -> constant angle `off`)
    src = [None] * nf
    mult = [0.0] * nf
    off = [0.0] * nf
    for j, ci in enumerate(ctrl):
        src[int(ci)] = j
        mult[int(ci)] = 1.0
        off[int(ci)] = 0.0
    # mimic reads post-ctrl pre-mimic values
    pre_src = list(src)
    pre_mult = list(mult)
    pre_off = list(off)
    for d, s, m, o in zip(mdst, msrc, mmul, moff):
        d, s = int(d), int(s)
        if pre_src[s] is not None:
            src[d] = pre_src[s]
            mult[d] = float(m) * pre_mult[s]
            off[d] = float(m) * pre_off[s] + float(o)
        else:
            src[d] = None
            mult[d] = 0.0
            off[d] = float(o)  # constant angle

    frames = []
    tables = np.zeros((nf, 4, 16), dtype=np.float64)

    def put(slot, n, mat):  # mat is (ni, 4)
        tables[n, slot, : ni * 4] = mat.reshape(-1)

    for n in range(nf):
        O4 = origins[n]
        A = O4[:ni, :].copy()
        t = int(types[n])
        if t == 1:  # revolute
            r = float(np.linalg.norm(axes[n]))
            if r < 1e-20:
                t = 0  # degenerate axis -> identity rotation
            else:
                K4 = np.zeros((4, 4))
                K4[:3, :3] = _skew(axes[n] / r)
                B = (O4 @ K4)[:ni, :]
                C = (O4 @ K4 @ K4)[:ni, :]
                if src[n] is None:
                    a = r * off[n]
                    put(0, n, A + np.sin(a) * B + (1.0 - np.cos(a)) * C)
                    frames.append(dict(kind="const"))
                else:
                    # L = A' + sin(x)*B + cos(x)*(-C)  with A' = A + C
                    put(0, n, A + C)
                    put(1, n, B)
                    put(2, n, -C)
                    tables[n, 3, 2] = np.pi / 2.0
                    frames.append(dict(kind="rev", src=src[n],
                                       mult=r * mult[n], off=r * off[n]))
                continue
        if t == 2:  # prismatic (raw, unnormalized axis)
            T4 = np.zeros((4, 4))
            T4[:3, 3] = axes[n]
            B = (O4 @ T4)[:ni, :]
            if src[n] is None:
                put(0, n, A + off[n] * B)
                frames.append(dict(kind="const"))
            else:
                put(0, n, A)
                put(1, n, B)
                frames.append(dict(kind="pri", src=src[n],
                                   mult=mult[n], off=off[n]))
            continue
        # fixed / degenerate
        put(0, n, A)
        frames.append(dict(kind="const"))

    return affine, frames, tables.astype(np.float32)


# --------------------------------------------------------------------------
# Device program
# --------------------------------------------------------------------------

def _build_program(b_core, dof, nf, affine, frames):
    """Builds the Bass/Tile program. Returns compiled Bacc."""
    assert b_core % P == 0
    q = b_core // P  # batch elements per partition (inner, contiguous)
    ni = 3 if affine else 4  # state rows
    nk = ni  # contraction extent in the chain product
    nchunks = (nf + CHUNK - 1) // CHUNK
    assert nf % CHUNK == 0

    nc = bacc.Bacc("TRN2", target_bir_lowering=False, debug=False)

    theta_d = nc.dram_tensor("theta", [b_core, dof], F32, kind="ExternalInput").ap()
    tables_d = nc.dram_tensor("tables", [P, nf, 4, 16], F32,
                              kind="ExternalInput").ap()
    out_d = nc.dram_tensor("out", [b_core, nf * 16], F32,
                           kind="ExternalOutput").ap()

    theta_v = theta_d.rearrange("(p q) d -> p q d", p=P)
    out_v = out_d.rearrange("(p q) (n e) -> p q n e", p=P, e=16)

    from contextlib import ExitStack

    reps = int(os.environ.get("FK_REPS", "1"))

    with tile.TileContext(nc) as tc, ExitStack() as ctx:
        pool = ctx.enter_context(tc.tile_pool(name="persist", bufs=1))
        lpool = ctx.enter_context(tc.tile_pool(name="lpool", bufs=4))
        mpool = ctx.enter_context(tc.tile_pool(name="mpool", bufs=4))

        theta_t = pool.tile([P, q, dof], F32)
        nc.sync.dma_start(theta_t[:], theta_v)

        tables_t = pool.tile([P, nf, 4, 16], F32)
        nc.sync.dma_start(tables_t[:], tables_d)

        u_t = pool.tile([P, nf, q], F32, tag="u_t")
        w_t = pool.tile([P, nf, q], F32, tag="w_t")

        # staging buffers (manual double buffer)
        stags = [pool.tile([P, q, CHUNK, 16], F32, tag=f"stag{i}",
                           name=f"stag{i}") for i in range(2)]
        if affine:
            for st in stags:
                nc.vector.memset(st[:, :, :, 12:15], 0.0)
                nc.vector.memset(st[:, :, :, 15], 1.0)

        for _rep in range(reps):
            # ---- coefficient planes ------------------------------------------
            # x = clamp(mult*theta+off, [-pi, pi]); u = sin(x); w = cos(x)
            # (cos computed as Sin(pi/2 - |x|) since the ScalarE Sin LUT only
            # accepts [-pi, pi]).
            pi = float(np.pi)
            op = mybir.AluOpType
            xpool = ctx.enter_context(tc.tile_pool(name="xpool", bufs=3))
            for n, fr in enumerate(frames):
                if fr["kind"] == "rev":
                    src_ap = theta_t[:, :, fr["src"]]
                    x_c = xpool.tile([P, q], F32, tag="xc")
                    nc.vector.tensor_scalar(x_c[:], src_ap, fr["mult"], fr["off"],
                                            op0=op.mult, op1=op.add)
                    nc.vector.tensor_scalar(x_c[:], x_c[:], pi, -pi,
                                            op0=op.min, op1=op.max)
                    nc.scalar.activation(u_t[:, n, :], x_c[:], AF.Sin)
                    a_x = xpool.tile([P, q], F32, tag="ax")
                    nc.scalar.activation(a_x[:], x_c[:], AF.Abs)
                    nc.scalar.activation(w_t[:, n, :], a_x[:], AF.Sin,
                                         bias=tables_t[:, n, 3, 2:3],
                                         scale=-1.0)
                elif fr["kind"] == "pri":
                    src_ap = theta_t[:, :, fr["src"]]
                    nc.vector.tensor_scalar(u_t[:, n, :], src_ap,
                                            fr["mult"], fr["off"],
                                            op0=op.mult, op1=op.add)

            # ---- helpers ------------------------------------------------------
            def tab(n, slot):
                # [P, 4(k), 4(j)] view of one table matrix
                return tables_t[:, n, slot, :].rearrange("p (k j) -> p k j", j=4)

            def tab_b(n, slot, nk_, nj_):
                # broadcast to [P, nk_, nj_, q] (steps 0 on q)
                a = tab(n, slot)[:, :nk_, :nj_]
                return a.unsqueeze(3).broadcast_to([P, nk_, nj_, q])

            def stag_view(ci, c):
                # [P, 4(i), 4(j), q] of staged frame transform
                return stags[ci][:, :, c, :] \
                    .rearrange("p q (i j) -> p q i j", j=4).transpose([0, 2, 3, 1])

            # ---- per-frame scan ----------------------------------------------
            prev = None  # (chunk_tile_idx, c)
            for n, fr in enumerate(frames):
                ci, c = (n // CHUNK) % 2, n % CHUNK
                out_f = stag_view(ci, c)  # [P,4,4,q]

                kind = fr["kind"]
                if kind == "const":
                    l_ap = None  # products read the table directly
                else:
                    l_t = lpool.tile([P, 16, q], F32, tag="L")
                    l_r = l_t[:].rearrange("p (k j) q -> p k j q", j=4)
                    ub = u_t[:, n, :].unsqueeze(1).unsqueeze(2) \
                        .broadcast_to([P, ni, 3, q])
                    if kind == "rev":
                        wb = w_t[:, n, :].unsqueeze(1).unsqueeze(2) \
                            .broadcast_to([P, ni, 3, q])
                        m_b = mpool.tile([P, ni, 3, q], F32, tag="mB")
                        m_c = mpool.tile([P, ni, 3, q], F32, tag="mC")
                        nc.gpsimd.tensor_mul(m_b[:], ub, tab_b(n, 1, ni, 3))
                        nc.gpsimd.tensor_mul(m_c[:], wb, tab_b(n, 2, ni, 3))
                        lr_s = mpool.tile([P, ni, 3, q], F32, tag="lrs")
                        nc.gpsimd.tensor_add(lr_s[:], m_b[:], m_c[:])
                        # L rotation block = sum + A'
                        nc.gpsimd.tensor_add(l_r[:, :ni, :3, :], lr_s[:],
                                             tab_b(n, 0, ni, 3))
                        # L translation column = A' col 3 (const)
                        a_col3 = tab(n, 0)[:, :ni, 3].unsqueeze(2) \
                            .broadcast_to([P, ni, q])
                        nc.scalar.copy(l_r[:, :ni, 3, :], a_col3)
                    else:  # prismatic: L = A + u*B ; B nonzero only in col 3
                        m_b = mpool.tile([P, ni, 1, q], F32, tag="mB")
                        ub1 = u_t[:, n, :].unsqueeze(1).unsqueeze(2) \
                            .broadcast_to([P, ni, 1, q])
                        nc.gpsimd.tensor_mul(
                            m_b[:], ub1,
                            tab(n, 1)[:, :ni, 3:4].unsqueeze(3)
                            .broadcast_to([P, ni, 1, q]))
                        nc.vector.tensor_add(
                            l_r[:, :ni, 3:4, :], m_b[:],
                            tab(n, 0)[:, :ni, 3:4].unsqueeze(3)
                            .broadcast_to([P, ni, 1, q]))
                        # rotation block is constant = A
                        a_rot = tab(n, 0)[:, :ni, :3].unsqueeze(3) \
                            .broadcast_to([P, ni, 3, q])
                        nc.scalar.copy(l_r[:, :ni, :3, :], a_rot)
                    l_ap = l_r

                def lrow(k):
                    # L row k broadcast over i: [P, ni, 4, q]
                    if l_ap is not None:
                        return l_ap[:, k, :, :].unsqueeze(1) \
                            .broadcast_to([P, ni, 4, q])
                    return tab(n, 0)[:, k, :].unsqueeze(1).unsqueeze(3) \
                        .broadcast_to([P, ni, 4, q])

                if prev is None:
                    # W_0 = L_0 : write directly into staging
                    if l_ap is not None:
                        nc.vector.tensor_copy(out_f[:, :ni, :, :], l_ap[:, :ni, :, :])
                    else:
                        nc.scalar.copy(
                            out_f[:, :ni, :, :],
                            tab(n, 0)[:, :ni, :].unsqueeze(3)
                            .broadcast_to([P, ni, 4, q]))
                else:
                    w_v = stag_view(*prev)  # [P,4,4,q] previous transform

                    def wcol(k):
                        return w_v[:, :ni, k, :].unsqueeze(2) \
                            .broadcast_to([P, ni, 4, q])

                    p0 = mpool.tile([P, ni, 4, q], F32, tag="p0")
                    p1 = mpool.tile([P, ni, 4, q], F32, tag="p1")
                    nc.vector.tensor_mul(p0[:], wcol(0), lrow(0))
                    nc.vector.tensor_mul(p1[:], wcol(1), lrow(1))
                    nc.vector.tensor_add(p0[:], p0[:], p1[:])
                    nc.vector.tensor_mul(p1[:], wcol(2), lrow(2))
                    if nk == 3:
                        nc.vector.tensor_add(out_f[:, :ni, :, :], p0[:], p1[:])
                        # affine fix: out[:, i, 3] += W[:, i, 3]
                        nc.vector.tensor_add(out_f[:, :ni, 3, :],
                                             out_f[:, :ni, 3, :],
                                             w_v[:, :ni, 3, :])
                    else:
                        nc.vector.tensor_add(p0[:], p0[:], p1[:])
                        nc.vector.tensor_mul(p1[:], wcol(3), lrow(3))
                        nc.vector.tensor_add(out_f[:, :ni, :, :], p0[:], p1[:])

                prev = (ci, c)

                # chunk complete -> DMA out
                if c == CHUNK - 1:
                    g = n // CHUNK
                    src = stags[ci][:].rearrange("p q c e -> p q (c e)")
                    dst = out_v[:, :, g * CHUNK:(g + 1) * CHUNK, :] \
                        .rearrange("p q c e -> p q (c e)")
                    nc.sync.dma_start(dst, src)

    nc.compile()
    return nc


def _get_program(b_core, dof, nf, affine, frames):
    key = (b_core, dof, nf, affine, os.environ.get("FK_REPS", "1"),
           tuple((f["kind"], f.get("src"), f.get("mult"), f.get("off"))
                 for f in frames))
    prog = _program_cache.get(key)
    if prog is None:
        prog = _build_program(b_core, dof, nf, affine, frames)
        _program_cache[key] = prog
    return prog


# --------------------------------------------------------------------------
# Entry point
# --------------------------------------------------------------------------

def kernel(joint_angles, all_axes, all_origins, mimic_multipliers,
           mimic_offsets, ctrlable_indices, mimic_dst_indices,
           mimic_src_indices, joint_types):
    global last_results

    theta = np.ascontiguousarray(np.asarray(joint_angles, dtype=np.float32))
    batch, dof = theta.shape
    nf = np.asarray(all_axes).shape[0]

    affine, frames, tables = _frame_specs(
        all_axes, all_origins, mimic_multipliers, mimic_offsets,
        ctrlable_indices, mimic_dst_indices, mimic_src_indices, joint_types)

    n_cores = N_CORES
    assert batch % n_cores == 0
    b_core = batch // n_cores

    nc = _get_program(b_core, dof, nf, affine, frames)

    tables_rep = np.ascontiguousarray(
        np.broadcast_to(tables[None], (P, nf, 4, 16)).astype(np.float32))
    global last_tables_rep
    last_tables_rep = tables_rep

    in_maps = []
    for i in range(n_cores):
        in_maps.append({
            "theta": np.ascontiguousarray(theta[i * b_core:(i + 1) * b_core]),
            "tables": tables_rep,
        })

    res = run_bass_kernel_spmd(nc, in_maps, core_ids=list(range(n_cores)))
    last_results = res

    out = np.concatenate([res.results[i]["out"] for i in range(n_cores)], axis=0)
    return out.reshape(batch, nf, 4, 4)

